# revision 19
# baseline (speedup 1.0000x reference)
"""BatchedMoE Trainium2 kernel.

Expert-parallel over 8 NeuronCores: host computes the (tiny) router +
top-2 dispatch in numpy; core c runs expert c's gated MLP over the
tokens routed to it (capacity-padded), plus the shared-expert MLP for
a 1/8 slice of all tokens. Matmuls run bf16 x bf16 with fp32 PSUM
accumulation; the silu/gating arithmetic stays fp32. Host
scatters/combines the partial outputs.

Self-contained: only numpy + concourse imports, no sibling files.
"""
import numpy as np

B, T, C = 4, 2048, 1024
E = 8            # experts == cores
KTOP = 2         # experts per token
H = 1408         # expert intermediate
HS = 2816        # shared intermediate
N = B * T        # 8192 tokens
TPC = N // 8     # tokens per core for the shared expert
KC = C // 128    # 8 k-tiles over C
NH = H // 128    # 11 h-tiles
NHS = HS // 128  # 22 hs-tiles
NC2 = C // 512   # 2 c-halves

TRACE = False
LAST_EXEC_NS = None
LAST_RESULTS = None

_cache = {}


def _build(cap):
    import concourse.bacc as bacc
    import concourse.tile as tile
    import concourse.mybir as mybir
    from contextlib import ExitStack

    f32 = mybir.dt.float32
    bf16 = mybir.dt.bfloat16
    AF = mybir.ActivationFunctionType

    nc = bacc.Bacc("TRN2", target_bir_lowering=False, debug=False)

    xdT = nc.dram_tensor("xdT", [C, cap], bf16, kind="ExternalInput").ap()
    w1 = nc.dram_tensor("w1", [C, H], bf16, kind="ExternalInput").ap()
    w2 = nc.dram_tensor("w2", [C, H], bf16, kind="ExternalInput").ap()
    w3 = nc.dram_tensor("w3", [H, C], bf16, kind="ExternalInput").ap()
    probs = nc.dram_tensor("probs", [128, cap], f32, kind="ExternalInput").ap()
    xsT = nc.dram_tensor("xsT", [C, TPC], bf16, kind="ExternalInput").ap()
    ws1b = nc.dram_tensor("ws1b", [NHS, 128, C], bf16, kind="ExternalInput").ap()
    ws2b = nc.dram_tensor("ws2b", [NHS, 128, C], bf16, kind="ExternalInput").ap()
    ws3 = nc.dram_tensor("ws3", [HS, C], bf16, kind="ExternalInput").ap()
    yd = nc.dram_tensor("yd", [cap, C], f32, kind="ExternalOutput").ap()
    ys = nc.dram_tensor("ys", [TPC, C], f32, kind="ExternalOutput").ap()

    groups = []
    s = 0
    while s < cap:
        w = min(512, cap - s)
        groups.append((s, w))
        s += w

    NTH = TPC // 512
    NB_EARLY = 3  # shared-expert steps run up front as PE filler while
                  # the phase-A weights stream in

    with tile.TileContext(nc) as tc:
        with ExitStack() as outer:
            # pools that live across phases (small)
            xsp = outer.enter_context(tc.tile_pool(name="xsB", bufs=1))
            cbp = outer.enter_context(tc.tile_pool(name="cbB", bufs=4))
            hep = outer.enter_context(tc.tile_pool(name="hsE", bufs=1))
            spB = outer.enter_context(tc.tile_pool(name="tmpB", bufs=2))

            xsb = [xsp.tile([128, TPC], bf16, tag=f"xs{k}", name=f"xsb{k}")
                   for k in range(KC)]
            hstE = [hep.tile([128, TPC], bf16, tag=f"hsE{j}", name=f"hstE{j}")
                    for j in range(NB_EARLY)]

            def b1_step(j, hst_tile, psum_pool, amortize, cbs=None):
                if cbs is not None:
                    cb1, cb2 = cbs
                else:
                    cb1 = cbp.tile([128, C], bf16, tag="cb1", name="cb1")
                    nc.sync.dma_start(cb1[:], ws1b[j, :, :])
                    cb2 = cbp.tile([128, C], bf16, tag="cb2", name="cb2")
                    nc.sync.dma_start(cb2[:], ws2b[j, :, :])
                if amortize:
                    p1 = [psum_pool.tile([128, 512], f32, tag=f"pB1_{th}",
                                         name=f"pB1_{th}") for th in range(NTH)]
                    p2 = [psum_pool.tile([128, 512], f32, tag=f"pB2_{th}",
                                         name=f"pB2_{th}") for th in range(NTH)]
                    for k in range(KC):
                        for th in range(NTH):
                            nc.tensor.matmul(
                                p1[th][:], cb1[:, k * 128:(k + 1) * 128],
                                xsb[k][:, th * 512:(th + 1) * 512],
                                start=(k == 0), stop=(k == KC - 1))
                    for k in range(KC):
                        for th in range(NTH):
                            nc.tensor.matmul(
                                p2[th][:], cb2[:, k * 128:(k + 1) * 128],
                                xsb[k][:, th * 512:(th + 1) * 512],
                                start=(k == 0), stop=(k == KC - 1))
                    for th in range(NTH):
                        sl = spB.tile([128, 512], f32, tag="slB", name="slB")
                        nc.scalar.activation(sl[:], p1[th][:], AF.Silu)
                        nc.vector.tensor_mul(
                            hst_tile[:, th * 512:(th + 1) * 512],
                            sl[:], p2[th][:])
                else:
                    # low-psum variant for the early filler steps
                    for th in range(NTH):
                        p1 = psum_pool.tile([128, 512], f32, tag="pE1", name="pE1")
                        for k in range(KC):
                            nc.tensor.matmul(
                                p1[:], cb1[:, k * 128:(k + 1) * 128],
                                xsb[k][:, th * 512:(th + 1) * 512],
                                start=(k == 0), stop=(k == KC - 1))
                        p2 = psum_pool.tile([128, 512], f32, tag="pE2", name="pE2")
                        for k in range(KC):
                            nc.tensor.matmul(
                                p2[:], cb2[:, k * 128:(k + 1) * 128],
                                xsb[k][:, th * 512:(th + 1) * 512],
                                start=(k == 0), stop=(k == KC - 1))
                        sl = spB.tile([128, 512], f32, tag="slB", name="slB")
                        nc.scalar.activation(sl[:], p1[:], AF.Silu)
                        nc.vector.tensor_mul(
                            hst_tile[:, th * 512:(th + 1) * 512], sl[:], p2[:])

            # ---- early B1 filler: covers the phase-A weight stream ----
            # j=0 weight blocks load before xsb so the first chain starts
            # after cb1 + xsb[0] instead of after the full xsb stream.
            with tc.tile_pool(name="psE", bufs=2, space="PSUM") as psE:
                cb1_0 = cbp.tile([128, C], bf16, tag="cb1", name="cb1")
                nc.sync.dma_start(cb1_0[:], ws1b[0, :, :])
                cb2_0 = cbp.tile([128, C], bf16, tag="cb2", name="cb2")
                nc.sync.dma_start(cb2_0[:], ws2b[0, :, :])
                for k in range(KC):
                    nc.sync.dma_start(xsb[k][:], xsT[k * 128:(k + 1) * 128, :])
                b1_step(0, hstE[0], psE, amortize=False, cbs=(cb1_0, cb2_0))
                for j in range(1, NB_EARLY):
                    b1_step(j, hstE[j], psE, amortize=False)

            # ---------------- Phase A: routed expert ----------------
            with ExitStack() as pa:
                wp = pa.enter_context(tc.tile_pool(name="wA", bufs=1))
                xp = pa.enter_context(tc.tile_pool(name="xA", bufs=2))
                hp = pa.enter_context(tc.tile_pool(name="hA", bufs=1))
                pp = pa.enter_context(tc.tile_pool(name="pbc", bufs=2))
                sp = pa.enter_context(tc.tile_pool(name="tmpA", bufs=2))
                op = pa.enter_context(tc.tile_pool(name="outA", bufs=2))
                psh = pa.enter_context(tc.tile_pool(name="psA", bufs=2, space="PSUM"))
                psy = pa.enter_context(tc.tile_pool(name="psyA", bufs=2, space="PSUM"))

                w1sb = [wp.tile([128, H], bf16, tag=f"w1_{k}", name=f"w1sb{k}")
                        for k in range(KC)]
                w2sb = [wp.tile([128, H], bf16, tag=f"w2_{k}", name=f"w2sb{k}")
                        for k in range(KC)]
                w3sb = [wp.tile([128, C], bf16, tag=f"w3_{h}", name=f"w3sb{h}")
                        for h in range(NH)]

                # group-0 x first (small), then weights k-interleaved in
                # first-use order; w3 trickles in behind w1/w2.
                g0s, g0w = groups[0]
                xg = [xp.tile([128, g0w], bf16, tag=f"x{k}", name=f"xg{k}")
                      for k in range(KC)]
                for k in range(KC):
                    nc.sync.dma_start(
                        xg[k][:], xdT[k * 128:(k + 1) * 128, g0s:g0s + g0w])
                    nc.sync.dma_start(w1sb[k][:], w1[k * 128:(k + 1) * 128, :])
                pb = pp.tile([128, g0w], f32, tag="pb")
                nc.sync.dma_start(pb[:], probs[:, g0s:g0s + g0w])
                for k in range(KC):
                    nc.sync.dma_start(w2sb[k][:], w2[k * 128:(k + 1) * 128, :])
                for h in range(NH):
                    nc.sync.dma_start(w3sb[h][:], w3[h * 128:(h + 1) * 128, :])

                for gi, (gs, gw) in enumerate(groups):
                    if gi > 0:
                        xg = [xp.tile([128, gw], bf16, tag=f"x{k}",
                                      name=f"xg{k}") for k in range(KC)]
                        for k in range(KC):
                            nc.sync.dma_start(
                                xg[k][:], xdT[k * 128:(k + 1) * 128, gs:gs + gw])
                        pb = pp.tile([128, gw], f32, tag="pb")
                        nc.sync.dma_start(pb[:], probs[:, gs:gs + gw])
                    hts = []
                    for h in range(NH):
                        p1 = psh.tile([128, gw], f32, tag="p1", name="p1")
                        for k in range(KC):
                            nc.tensor.matmul(
                                p1[:], w1sb[k][:, h * 128:(h + 1) * 128],
                                xg[k][:], start=(k == 0), stop=(k == KC - 1))
                        p2 = psh.tile([128, gw], f32, tag="p2", name="p2")
                        for k in range(KC):
                            nc.tensor.matmul(
                                p2[:], w2sb[k][:, h * 128:(h + 1) * 128],
                                xg[k][:], start=(k == 0), stop=(k == KC - 1))
                        sl = sp.tile([128, gw], f32, tag="sl", name="sl")
                        nc.scalar.activation(sl[:], p1[:], AF.Silu)
                        t2 = sp.tile([128, gw], f32, tag="t2", name="t2")
                        nc.vector.tensor_mul(t2[:], p2[:], pb[:])
                        ht = hp.tile([128, gw], bf16, tag=f"h{h}", name=f"ht{h}")
                        nc.vector.tensor_mul(ht[:], sl[:], t2[:])
                        hts.append(ht)

                    for t in range(gw // 128):
                        py = [psy.tile([128, 512], f32, tag=f"py{c}",
                                       name=f"py{c}") for c in range(NC2)]
                        for h in range(NH):
                            for c in range(NC2):
                                nc.tensor.matmul(
                                    py[c][:], hts[h][:, t * 128:(t + 1) * 128],
                                    w3sb[h][:, c * 512:(c + 1) * 512],
                                    start=(h == 0), stop=(h == NH - 1))
                        for c in range(NC2):
                            ot = op.tile([128, 512], f32, tag="ot", name="ot")
                            nc.vector.tensor_copy(ot[:], py[c][:])
                            nc.sync.dma_start(
                                yd[gs + t * 128: gs + (t + 1) * 128,
                                   c * 512:(c + 1) * 512], ot[:])

            # ---------------- Phase B: shared expert (rest) ----------------
            with ExitStack() as pbx:
                hbp = pbx.enter_context(tc.tile_pool(name="hsB", bufs=1))
                w3sp = pbx.enter_context(tc.tile_pool(name="ws3B", bufs=1))
                oB = pbx.enter_context(tc.tile_pool(name="outB", bufs=2))
                psB = pbx.enter_context(tc.tile_pool(name="psB", bufs=1, space="PSUM"))
                psyB = pbx.enter_context(tc.tile_pool(name="psyB", bufs=2, space="PSUM"))

                hst = hstE + [
                    hbp.tile([128, TPC], bf16, tag=f"hs{j}", name=f"hst{j}")
                    for j in range(NB_EARLY, NHS)]
                ws3sb = [w3sp.tile([128, C], bf16, tag=f"ws3_{j}", name=f"ws3sb{j}")
                         for j in range(NHS)]

                for j in range(NB_EARLY, NHS):
                    # pace the B2 weight prefetch: one slab per j step
                    nc.sync.dma_start(ws3sb[j][:], ws3[j * 128:(j + 1) * 128, :])
                    b1_step(j, hst[j], psB, amortize=True)
                for j in range(NB_EARLY):
                    nc.sync.dma_start(ws3sb[j][:], ws3[j * 128:(j + 1) * 128, :])

                for t in range(TPC // 128):
                    py = [psyB.tile([128, 512], f32, tag=f"pyB{c}",
                                    name=f"pyB{c}") for c in range(NC2)]
                    for j in range(NHS):
                        for c in range(NC2):
                            nc.tensor.matmul(
                                py[c][:], hst[j][:, t * 128:(t + 1) * 128],
                                ws3sb[j][:, c * 512:(c + 1) * 512],
                                start=(j == 0), stop=(j == NHS - 1))
                    for c in range(NC2):
                        ot = oB.tile([128, 512], f32, tag="otB", name="otB")
                        nc.vector.tensor_copy(ot[:], py[c][:])
                        nc.sync.dma_start(
                            ys[t * 128:(t + 1) * 128,
                               c * 512:(c + 1) * 512], ot[:])

    nc.compile()
    return nc


def _get_nc(cap):
    if cap not in _cache:
        _cache[cap] = _build(cap)
    return _cache[cap]


def kernel(x, Wg, W1, W2, W3, Ws1, Ws2, Ws3):
    global LAST_EXEC_NS, LAST_RESULTS
    from concourse import bass_utils
    import ml_dtypes

    bf = ml_dtypes.bfloat16
    x = np.ascontiguousarray(np.asarray(x, dtype=np.float32))
    Wg = np.asarray(Wg, dtype=np.float32)
    W1 = np.asarray(W1, dtype=np.float32)
    W2 = np.asarray(W2, dtype=np.float32)
    W3 = np.asarray(W3, dtype=np.float32)
    Ws1 = np.asarray(Ws1, dtype=np.float32)
    Ws2 = np.asarray(Ws2, dtype=np.float32)
    Ws3 = np.asarray(Ws3, dtype=np.float32)

    xf = x.reshape(N, C)

    # ---- router + top-2 + softmax (fp32, matches jax.lax.top_k tie-break) ----
    router = xf @ Wg                                   # [N, E]
    i0 = np.argmax(router, axis=1)
    ar = np.arange(N)
    l0 = router[ar, i0]
    r2 = router.copy()
    r2[ar, i0] = -np.inf
    i1 = np.argmax(r2, axis=1)
    l1 = router[ar, i1]
    m = np.maximum(l0, l1)
    e0 = np.exp(l0 - m)
    e1 = np.exp(l1 - m)
    zs = e0 + e1
    p0 = (e0 / zs).astype(np.float32)
    p1 = (e1 / zs).astype(np.float32)

    # ---- dispatch: sort (token, slot) pairs by expert ----
    flat_e = np.concatenate([i0, i1])                  # [2N]
    flat_t = np.concatenate([ar, ar])
    flat_p = np.concatenate([p0, p1])
    order = np.argsort(flat_e, kind="stable")
    counts = np.bincount(flat_e, minlength=E)
    offs = np.zeros(E + 1, dtype=np.int64)
    np.cumsum(counts, out=offs[1:])

    cap = max(2304, int(-(-counts.max() // 256) * 256))

    # slot of each pair inside its expert's buffer
    slot = np.empty(2 * N, dtype=np.int64)
    slot[order] = np.arange(2 * N) - offs[flat_e[order]]
    gslot = flat_e * cap + slot                        # into stacked [E*cap, C]

    # ---- per-core inputs ----
    def blk(w, nblocks):
        return np.ascontiguousarray(
            w.reshape(KC, 128, nblocks, 128).transpose(2, 1, 0, 3)
            .reshape(nblocks, 128, C).astype(bf))

    ws1b = blk(Ws1, NHS)
    ws2b = blk(Ws2, NHS)
    ws3_bf = np.ascontiguousarray(Ws3.astype(bf))
    xfb = xf.astype(bf)

    in_maps = []
    for e in range(E):
        sel = order[offs[e]:offs[e + 1]]
        toks = flat_t[sel]
        pr = flat_p[sel]
        xd = np.zeros((cap, C), dtype=bf)
        xd[:len(toks)] = xfb[toks]
        pbc = np.zeros((cap,), dtype=np.float32)
        pbc[:len(toks)] = pr
        in_maps.append({
            "xdT": np.ascontiguousarray(xd.T),
            "w1": np.ascontiguousarray(W1[e].astype(bf)),
            "w2": np.ascontiguousarray(W2[e].astype(bf)),
            "w3": np.ascontiguousarray(W3[e].astype(bf)),
            "probs": np.ascontiguousarray(np.broadcast_to(pbc, (128, cap))),
            "xsT": np.ascontiguousarray(xfb[e * TPC:(e + 1) * TPC].T),
            "ws1b": ws1b,
            "ws2b": ws2b,
            "ws3": ws3_bf,
        })

    nc = _get_nc(cap)
    res = None
    for attempt in range(3):
        try:
            res = bass_utils.run_bass_kernel_spmd(
                nc, in_maps, core_ids=list(range(8)), trace=TRACE)
            break
        except Exception:
            if attempt == 2:
                raise
    LAST_EXEC_NS = res.exec_time_ns
    LAST_RESULTS = res

    # ---- combine ----
    YD = np.concatenate([res.results[e]["yd"] for e in range(E)], axis=0)
    y = YD[gslot[:N]] + YD[gslot[N:]]
    y += np.concatenate([res.results[c]["ys"] for c in range(E)], axis=0)
    return y.reshape(B, T, C)


# revision 20
# speedup vs baseline: 1.0016x; 1.0016x over previous
"""BatchedMoE Trainium2 kernel.

Expert-parallel over 8 NeuronCores: host computes the (tiny) router +
top-2 dispatch in numpy; core c runs expert c's gated MLP over the
tokens routed to it (capacity-padded), plus the shared-expert MLP for
a 1/8 slice of all tokens. Matmuls run bf16 x bf16 with fp32 PSUM
accumulation; the silu/gating arithmetic stays fp32. Host
scatters/combines the partial outputs.

Self-contained: only numpy + concourse imports, no sibling files.
"""
import numpy as np

B, T, C = 4, 2048, 1024
E = 8            # experts == cores
KTOP = 2         # experts per token
H = 1408         # expert intermediate
HS = 2816        # shared intermediate
N = B * T        # 8192 tokens
TPC = N // 8     # tokens per core for the shared expert
KC = C // 128    # 8 k-tiles over C
NH = H // 128    # 11 h-tiles
NHS = HS // 128  # 22 hs-tiles
NC2 = C // 512   # 2 c-halves

TRACE = False
LAST_EXEC_NS = None
LAST_RESULTS = None

_cache = {}


def _build(cap):
    import concourse.bacc as bacc
    import concourse.tile as tile
    import concourse.mybir as mybir
    from contextlib import ExitStack

    f32 = mybir.dt.float32
    bf16 = mybir.dt.bfloat16
    AF = mybir.ActivationFunctionType

    nc = bacc.Bacc("TRN2", target_bir_lowering=False, debug=False)

    xdT = nc.dram_tensor("xdT", [C, cap], bf16, kind="ExternalInput").ap()
    w1 = nc.dram_tensor("w1", [C, H], bf16, kind="ExternalInput").ap()
    w2 = nc.dram_tensor("w2", [C, H], bf16, kind="ExternalInput").ap()
    w3 = nc.dram_tensor("w3", [H, C], bf16, kind="ExternalInput").ap()
    probs = nc.dram_tensor("probs", [128, cap], f32, kind="ExternalInput").ap()
    xsT = nc.dram_tensor("xsT", [C, TPC], bf16, kind="ExternalInput").ap()
    ws1b = nc.dram_tensor("ws1b", [NHS, 128, C], bf16, kind="ExternalInput").ap()
    ws2b = nc.dram_tensor("ws2b", [NHS, 128, C], bf16, kind="ExternalInput").ap()
    ws3 = nc.dram_tensor("ws3", [HS, C], bf16, kind="ExternalInput").ap()
    yd = nc.dram_tensor("yd", [cap, C], f32, kind="ExternalOutput").ap()
    ys = nc.dram_tensor("ys", [TPC, C], f32, kind="ExternalOutput").ap()

    groups = []
    s = 0
    while s < cap:
        w = min(512, cap - s)
        groups.append((s, w))
        s += w

    NTH = TPC // 512
    NB_EARLY = 4  # shared-expert steps run up front as PE filler while
                  # the phase-A weights stream in

    with tile.TileContext(nc) as tc:
        with ExitStack() as outer:
            # pools that live across phases (small)
            xsp = outer.enter_context(tc.tile_pool(name="xsB", bufs=1))
            cbp = outer.enter_context(tc.tile_pool(name="cbB", bufs=4))
            hep = outer.enter_context(tc.tile_pool(name="hsE", bufs=1))
            spB = outer.enter_context(tc.tile_pool(name="tmpB", bufs=2))

            xsb = [xsp.tile([128, TPC], bf16, tag=f"xs{k}", name=f"xsb{k}")
                   for k in range(KC)]
            hstE = [hep.tile([128, TPC], bf16, tag=f"hsE{j}", name=f"hstE{j}")
                    for j in range(NB_EARLY)]

            def b1_step(j, hst_tile, psum_pool, amortize, cbs=None):
                if cbs is not None:
                    cb1, cb2 = cbs
                else:
                    cb1 = cbp.tile([128, C], bf16, tag="cb1", name="cb1")
                    nc.sync.dma_start(cb1[:], ws1b[j, :, :])
                    cb2 = cbp.tile([128, C], bf16, tag="cb2", name="cb2")
                    nc.sync.dma_start(cb2[:], ws2b[j, :, :])
                if amortize:
                    p1 = [psum_pool.tile([128, 512], f32, tag=f"pB1_{th}",
                                         name=f"pB1_{th}") for th in range(NTH)]
                    p2 = [psum_pool.tile([128, 512], f32, tag=f"pB2_{th}",
                                         name=f"pB2_{th}") for th in range(NTH)]
                    for k in range(KC):
                        for th in range(NTH):
                            nc.tensor.matmul(
                                p1[th][:], cb1[:, k * 128:(k + 1) * 128],
                                xsb[k][:, th * 512:(th + 1) * 512],
                                start=(k == 0), stop=(k == KC - 1))
                    for k in range(KC):
                        for th in range(NTH):
                            nc.tensor.matmul(
                                p2[th][:], cb2[:, k * 128:(k + 1) * 128],
                                xsb[k][:, th * 512:(th + 1) * 512],
                                start=(k == 0), stop=(k == KC - 1))
                    for th in range(NTH):
                        sl = spB.tile([128, 512], f32, tag="slB", name="slB")
                        nc.scalar.activation(sl[:], p1[th][:], AF.Silu)
                        nc.vector.tensor_mul(
                            hst_tile[:, th * 512:(th + 1) * 512],
                            sl[:], p2[th][:])
                else:
                    # low-psum variant for the early filler steps
                    for th in range(NTH):
                        p1 = psum_pool.tile([128, 512], f32, tag="pE1", name="pE1")
                        for k in range(KC):
                            nc.tensor.matmul(
                                p1[:], cb1[:, k * 128:(k + 1) * 128],
                                xsb[k][:, th * 512:(th + 1) * 512],
                                start=(k == 0), stop=(k == KC - 1))
                        p2 = psum_pool.tile([128, 512], f32, tag="pE2", name="pE2")
                        for k in range(KC):
                            nc.tensor.matmul(
                                p2[:], cb2[:, k * 128:(k + 1) * 128],
                                xsb[k][:, th * 512:(th + 1) * 512],
                                start=(k == 0), stop=(k == KC - 1))
                        sl = spB.tile([128, 512], f32, tag="slB", name="slB")
                        nc.scalar.activation(sl[:], p1[:], AF.Silu)
                        nc.vector.tensor_mul(
                            hst_tile[:, th * 512:(th + 1) * 512], sl[:], p2[:])

            # ---- early B1 filler: covers the phase-A weight stream ----
            # j=0 weight blocks load before xsb so the first chain starts
            # after cb1 + xsb[0] instead of after the full xsb stream.
            with tc.tile_pool(name="psE", bufs=2, space="PSUM") as psE:
                cb1_0 = cbp.tile([128, C], bf16, tag="cb1", name="cb1")
                nc.sync.dma_start(cb1_0[:], ws1b[0, :, :])
                cb2_0 = cbp.tile([128, C], bf16, tag="cb2", name="cb2")
                nc.sync.dma_start(cb2_0[:], ws2b[0, :, :])
                for k in range(KC):
                    nc.sync.dma_start(xsb[k][:], xsT[k * 128:(k + 1) * 128, :])
                b1_step(0, hstE[0], psE, amortize=False, cbs=(cb1_0, cb2_0))
                for j in range(1, NB_EARLY):
                    b1_step(j, hstE[j], psE, amortize=False)

            # ---------------- Phase A: routed expert ----------------
            with ExitStack() as pa:
                wp = pa.enter_context(tc.tile_pool(name="wA", bufs=1))
                xp = pa.enter_context(tc.tile_pool(name="xA", bufs=2))
                hp = pa.enter_context(tc.tile_pool(name="hA", bufs=1))
                pp = pa.enter_context(tc.tile_pool(name="pbc", bufs=2))
                sp = pa.enter_context(tc.tile_pool(name="tmpA", bufs=2))
                op = pa.enter_context(tc.tile_pool(name="outA", bufs=2))
                psh = pa.enter_context(tc.tile_pool(name="psA", bufs=2, space="PSUM"))
                psy = pa.enter_context(tc.tile_pool(name="psyA", bufs=2, space="PSUM"))

                w1sb = [wp.tile([128, H], bf16, tag=f"w1_{k}", name=f"w1sb{k}")
                        for k in range(KC)]
                w2sb = [wp.tile([128, H], bf16, tag=f"w2_{k}", name=f"w2sb{k}")
                        for k in range(KC)]
                w3sb = [wp.tile([128, C], bf16, tag=f"w3_{h}", name=f"w3sb{h}")
                        for h in range(NH)]

                # group-0 x first (small), then weights k-interleaved in
                # first-use order; w3 trickles in behind w1/w2.
                g0s, g0w = groups[0]
                xg = [xp.tile([128, g0w], bf16, tag=f"x{k}", name=f"xg{k}")
                      for k in range(KC)]
                for k in range(KC):
                    nc.sync.dma_start(
                        xg[k][:], xdT[k * 128:(k + 1) * 128, g0s:g0s + g0w])
                    nc.sync.dma_start(w1sb[k][:], w1[k * 128:(k + 1) * 128, :])
                pb = pp.tile([128, g0w], f32, tag="pb")
                nc.sync.dma_start(pb[:], probs[:, g0s:g0s + g0w])
                for k in range(KC):
                    nc.sync.dma_start(w2sb[k][:], w2[k * 128:(k + 1) * 128, :])
                for h in range(NH):
                    nc.sync.dma_start(w3sb[h][:], w3[h * 128:(h + 1) * 128, :])

                for gi, (gs, gw) in enumerate(groups):
                    if gi > 0:
                        xg = [xp.tile([128, gw], bf16, tag=f"x{k}",
                                      name=f"xg{k}") for k in range(KC)]
                        for k in range(KC):
                            nc.sync.dma_start(
                                xg[k][:], xdT[k * 128:(k + 1) * 128, gs:gs + gw])
                        pb = pp.tile([128, gw], f32, tag="pb")
                        nc.sync.dma_start(pb[:], probs[:, gs:gs + gw])
                    hts = []
                    for h in range(NH):
                        p1 = psh.tile([128, gw], f32, tag="p1", name="p1")
                        for k in range(KC):
                            nc.tensor.matmul(
                                p1[:], w1sb[k][:, h * 128:(h + 1) * 128],
                                xg[k][:], start=(k == 0), stop=(k == KC - 1))
                        p2 = psh.tile([128, gw], f32, tag="p2", name="p2")
                        for k in range(KC):
                            nc.tensor.matmul(
                                p2[:], w2sb[k][:, h * 128:(h + 1) * 128],
                                xg[k][:], start=(k == 0), stop=(k == KC - 1))
                        sl = sp.tile([128, gw], f32, tag="sl", name="sl")
                        nc.scalar.activation(sl[:], p1[:], AF.Silu)
                        t2 = sp.tile([128, gw], f32, tag="t2", name="t2")
                        nc.vector.tensor_mul(t2[:], p2[:], pb[:])
                        ht = hp.tile([128, gw], bf16, tag=f"h{h}", name=f"ht{h}")
                        nc.vector.tensor_mul(ht[:], sl[:], t2[:])
                        hts.append(ht)

                    for t in range(gw // 128):
                        py = [psy.tile([128, 512], f32, tag=f"py{c}",
                                       name=f"py{c}") for c in range(NC2)]
                        for h in range(NH):
                            for c in range(NC2):
                                nc.tensor.matmul(
                                    py[c][:], hts[h][:, t * 128:(t + 1) * 128],
                                    w3sb[h][:, c * 512:(c + 1) * 512],
                                    start=(h == 0), stop=(h == NH - 1))
                        for c in range(NC2):
                            ot = op.tile([128, 512], f32, tag="ot", name="ot")
                            nc.vector.tensor_copy(ot[:], py[c][:])
                            nc.sync.dma_start(
                                yd[gs + t * 128: gs + (t + 1) * 128,
                                   c * 512:(c + 1) * 512], ot[:])

            # ---------------- Phase B: shared expert (rest) ----------------
            with ExitStack() as pbx:
                hbp = pbx.enter_context(tc.tile_pool(name="hsB", bufs=1))
                w3sp = pbx.enter_context(tc.tile_pool(name="ws3B", bufs=1))
                oB = pbx.enter_context(tc.tile_pool(name="outB", bufs=2))
                psB = pbx.enter_context(tc.tile_pool(name="psB", bufs=1, space="PSUM"))
                psyB = pbx.enter_context(tc.tile_pool(name="psyB", bufs=2, space="PSUM"))

                hst = hstE + [
                    hbp.tile([128, TPC], bf16, tag=f"hs{j}", name=f"hst{j}")
                    for j in range(NB_EARLY, NHS)]
                ws3sb = [w3sp.tile([128, C], bf16, tag=f"ws3_{j}", name=f"ws3sb{j}")
                         for j in range(NHS)]

                for j in range(NB_EARLY, NHS):
                    # pace the B2 weight prefetch: one slab per j step
                    nc.sync.dma_start(ws3sb[j][:], ws3[j * 128:(j + 1) * 128, :])
                    b1_step(j, hst[j], psB, amortize=True)
                for j in range(NB_EARLY):
                    nc.sync.dma_start(ws3sb[j][:], ws3[j * 128:(j + 1) * 128, :])

                for t in range(TPC // 128):
                    py = [psyB.tile([128, 512], f32, tag=f"pyB{c}",
                                    name=f"pyB{c}") for c in range(NC2)]
                    for j in range(NHS):
                        for c in range(NC2):
                            nc.tensor.matmul(
                                py[c][:], hst[j][:, t * 128:(t + 1) * 128],
                                ws3sb[j][:, c * 512:(c + 1) * 512],
                                start=(j == 0), stop=(j == NHS - 1))
                    for c in range(NC2):
                        ot = oB.tile([128, 512], f32, tag="otB", name="otB")
                        nc.vector.tensor_copy(ot[:], py[c][:])
                        nc.sync.dma_start(
                            ys[t * 128:(t + 1) * 128,
                               c * 512:(c + 1) * 512], ot[:])

    nc.compile()
    return nc


def _get_nc(cap):
    if cap not in _cache:
        _cache[cap] = _build(cap)
    return _cache[cap]


def kernel(x, Wg, W1, W2, W3, Ws1, Ws2, Ws3):
    global LAST_EXEC_NS, LAST_RESULTS
    from concourse import bass_utils
    import ml_dtypes

    bf = ml_dtypes.bfloat16
    x = np.ascontiguousarray(np.asarray(x, dtype=np.float32))
    Wg = np.asarray(Wg, dtype=np.float32)
    W1 = np.asarray(W1, dtype=np.float32)
    W2 = np.asarray(W2, dtype=np.float32)
    W3 = np.asarray(W3, dtype=np.float32)
    Ws1 = np.asarray(Ws1, dtype=np.float32)
    Ws2 = np.asarray(Ws2, dtype=np.float32)
    Ws3 = np.asarray(Ws3, dtype=np.float32)

    xf = x.reshape(N, C)

    # ---- router + top-2 + softmax (fp32, matches jax.lax.top_k tie-break) ----
    router = xf @ Wg                                   # [N, E]
    i0 = np.argmax(router, axis=1)
    ar = np.arange(N)
    l0 = router[ar, i0]
    r2 = router.copy()
    r2[ar, i0] = -np.inf
    i1 = np.argmax(r2, axis=1)
    l1 = router[ar, i1]
    m = np.maximum(l0, l1)
    e0 = np.exp(l0 - m)
    e1 = np.exp(l1 - m)
    zs = e0 + e1
    p0 = (e0 / zs).astype(np.float32)
    p1 = (e1 / zs).astype(np.float32)

    # ---- dispatch: sort (token, slot) pairs by expert ----
    flat_e = np.concatenate([i0, i1])                  # [2N]
    flat_t = np.concatenate([ar, ar])
    flat_p = np.concatenate([p0, p1])
    order = np.argsort(flat_e, kind="stable")
    counts = np.bincount(flat_e, minlength=E)
    offs = np.zeros(E + 1, dtype=np.int64)
    np.cumsum(counts, out=offs[1:])

    cap = max(2304, int(-(-counts.max() // 256) * 256))

    # slot of each pair inside its expert's buffer
    slot = np.empty(2 * N, dtype=np.int64)
    slot[order] = np.arange(2 * N) - offs[flat_e[order]]
    gslot = flat_e * cap + slot                        # into stacked [E*cap, C]

    # ---- per-core inputs ----
    def blk(w, nblocks):
        return np.ascontiguousarray(
            w.reshape(KC, 128, nblocks, 128).transpose(2, 1, 0, 3)
            .reshape(nblocks, 128, C).astype(bf))

    ws1b = blk(Ws1, NHS)
    ws2b = blk(Ws2, NHS)
    ws3_bf = np.ascontiguousarray(Ws3.astype(bf))
    xfb = xf.astype(bf)

    in_maps = []
    for e in range(E):
        sel = order[offs[e]:offs[e + 1]]
        toks = flat_t[sel]
        pr = flat_p[sel]
        xd = np.zeros((cap, C), dtype=bf)
        xd[:len(toks)] = xfb[toks]
        pbc = np.zeros((cap,), dtype=np.float32)
        pbc[:len(toks)] = pr
        in_maps.append({
            "xdT": np.ascontiguousarray(xd.T),
            "w1": np.ascontiguousarray(W1[e].astype(bf)),
            "w2": np.ascontiguousarray(W2[e].astype(bf)),
            "w3": np.ascontiguousarray(W3[e].astype(bf)),
            "probs": np.ascontiguousarray(np.broadcast_to(pbc, (128, cap))),
            "xsT": np.ascontiguousarray(xfb[e * TPC:(e + 1) * TPC].T),
            "ws1b": ws1b,
            "ws2b": ws2b,
            "ws3": ws3_bf,
        })

    nc = _get_nc(cap)
    res = None
    for attempt in range(3):
        try:
            res = bass_utils.run_bass_kernel_spmd(
                nc, in_maps, core_ids=list(range(8)), trace=TRACE)
            break
        except Exception:
            if attempt == 2:
                raise
    LAST_EXEC_NS = res.exec_time_ns
    LAST_RESULTS = res

    # ---- combine ----
    YD = np.concatenate([res.results[e]["yd"] for e in range(E)], axis=0)
    y = YD[gslot[:N]] + YD[gslot[N:]]
    y += np.concatenate([res.results[c]["ys"] for c in range(E)], axis=0)
    return y.reshape(B, T, C)
